# revision 1
# baseline (speedup 1.0000x reference)
"""Trainium2 Bass kernel for channel-attention + 2 residual conv blocks.

Data-parallel over batch (8 cores, 1 batch element each). Two SPMD launches:
  A) accumulate S_big = [q|k]^T [q|k] over all 65536 pixels (channel Gram,
     includes q/k norms on diagonal blocks; conv biases folded via ones-row).
  B) folded attention-v projection (Wav = softmax-attn @ Wv) + 4 3x3 convs
     (9 shifted-view matmuls each, pitch-258 zero-gap row layout) + residuals.
Host does only the O(64^2) softmax/fold algebra between launches.
"""
import sys, os
for p in ('/opt/trn_rl_repo', os.path.expanduser('~/.axon_site/_ro/trn_rl_repo')):
    if os.path.isdir(p) and p not in sys.path:
        sys.path.insert(0, p)

import numpy as np
import ml_dtypes
import concourse.bass as bass
import concourse.bacc as bacc
import concourse.tile as tile
from concourse import mybir
from concourse.bass_utils import run_bass_kernel_spmd

dt = mybir.dt
F32, BF16 = dt.float32, dt.bfloat16
BF = ml_dtypes.bfloat16
AF = mybir.ActivationFunctionType
OP = mybir.AluOpType

D = 64
HW = 65536
H = W_IMG = 256
NCORES = 8
R = 32  # rows per block in pass B


def _build_nc_a():
    nc = bacc.Bacc("TRN2", target_bir_lowering=False, debug=False)
    xa = nc.dram_tensor("xa", [D + 1, HW], BF16, kind="ExternalInput").ap()
    ya = nc.dram_tensor("ya", [D + 1, HW], BF16, kind="ExternalInput").ap()
    wq = nc.dram_tensor("wq", [D + 1, D], BF16, kind="ExternalInput").ap()
    wk = nc.dram_tensor("wk", [D + 1, D], BF16, kind="ExternalInput").ap()
    sbig = nc.dram_tensor("sbig", [128, 128], F32, kind="ExternalOutput").ap()

    CH = 4096          # pixels per DMA chunk
    NCH = HW // CH     # 16
    NIT = CH // 128    # 32 inner steps

    with tile.TileContext(nc) as tc:
        with tc.tile_pool(name="consts", bufs=1) as consts, \
             tc.tile_pool(name="io", bufs=3) as io, \
             tc.tile_pool(name="work", bufs=4) as work, \
             tc.tile_pool(name="qkps", bufs=4, space="PSUM") as qkps, \
             tc.tile_pool(name="accps", bufs=1, space="PSUM") as accps:
            wqt = consts.tile([D + 1, D], BF16)
            wkt = consts.tile([D + 1, D], BF16)
            nc.sync.dma_start(out=wqt, in_=wq)
            nc.sync.dma_start(out=wkt, in_=wk)
            sb = accps.tile([128, 128], F32)
            it = 0
            for c in range(NCH):
                xt = io.tile([D + 1, CH], BF16, tag="xt")
                yt = io.tile([D + 1, CH], BF16, tag="yt")
                nc.sync.dma_start(out=xt, in_=xa[:, c * CH:(c + 1) * CH])
                nc.sync.dma_start(out=yt, in_=ya[:, c * CH:(c + 1) * CH])
                for j in range(NIT):
                    qk_ps = qkps.tile([128, 128], F32)
                    nc.tensor.matmul(qk_ps[:, 0:64], xt[:, j * 128:(j + 1) * 128],
                                     wqt, start=True, stop=True)
                    nc.tensor.matmul(qk_ps[:, 64:128], yt[:, j * 128:(j + 1) * 128],
                                     wkt, start=True, stop=True)
                    qk = work.tile([128, 128], BF16)
                    nc.scalar.activation(out=qk[:, 0:64], in_=qk_ps[:, 0:64],
                                         func=AF.Copy, bias=0.0, scale=1.0)
                    nc.vector.tensor_copy(out=qk[:, 64:128], in_=qk_ps[:, 64:128])
                    nc.tensor.matmul(sb, qk, qk, start=(it == 0), stop=(it == NCH * NIT - 1),
                                     skip_group_check=True)
                    it += 1
            sout = work.tile([128, 128], F32, tag="sout")
            nc.vector.tensor_copy(out=sout, in_=sb)
            nc.sync.dma_start(out=sbig, in_=sout)
    nc.compile()
    return nc


def _build_nc_b():
    nc = bacc.Bacc("TRN2", target_bir_lowering=False, debug=False)
    yb = nc.dram_tensor("yb", [D, HW], BF16, kind="ExternalInput").ap()
    yf = nc.dram_tensor("yf", [D, HW], F32, kind="ExternalInput").ap()
    wavt = nc.dram_tensor("wavt", [D, D], BF16, kind="ExternalInput").ap()
    bav = nc.dram_tensor("bav", [D, 1], F32, kind="ExternalInput").ap()
    wt_d = [(nc.dram_tensor(f"wp{i}", [2 * D, 3 * D], BF16, kind="ExternalInput").ap(),
             nc.dram_tensor(f"w2{i}", [2 * D, 3 * D], BF16, kind="ExternalInput").ap())
            for i in range(1, 5)]
    bt_d = [nc.dram_tensor(f"b{i}", [D, 1], F32, kind="ExternalInput").ap()
            for i in range(1, 5)]
    out = nc.dram_tensor("out", [D, HW], F32, kind="ExternalOutput").ap()

    def extend_even(lo, hi):
        lo, hi = max(0, lo), min(256, hi)
        if (hi - lo) % 2:
            if hi < 256:
                hi += 1
            else:
                lo -= 1
        return lo, hi

    with tile.TileContext(nc) as tc:
        with tile_pools(tc) as (consts, ybp, stage0, stage, small, ps):
            wavt_t = consts.tile([D, D], BF16)
            bav_t = consts.tile([D, 1], F32)
            nc.sync.dma_start(out=wavt_t, in_=wavt)
            nc.sync.dma_start(out=bav_t, in_=bav)
            wts, bts = [], []
            for i in range(4):
                wp = consts.tile([2 * D, 3 * D], BF16, tag=f"wp{i}")
                w2 = consts.tile([2 * D, 3 * D], BF16, tag=f"w2{i}")
                b = consts.tile([D, 1], F32, tag=f"b{i}")
                nc.sync.dma_start(out=wp, in_=wt_d[i][0])
                nc.sync.dma_start(out=w2, in_=wt_d[i][1])
                nc.sync.dma_start(out=b, in_=bt_d[i])
                wts.append((wp, w2))
                bts.append(b)

            for blk in range(256 // R):
                r0, r1 = blk * R, (blk + 1) * R
                c3_lo, c3_hi = extend_even(r0 - 1, r1 + 1)
                o3_lo, o3_hi = extend_even(c3_lo - 1, c3_hi + 1)
                c1_lo, c1_hi = extend_even(o3_lo - 1, o3_hi + 1)
                t0_lo, t0_hi = extend_even(c1_lo - 1, c1_hi + 1)

                def mkstage(tag, lo, hi, pool=None):
                    vlo = 1 if lo == 0 else 0
                    vhi = 1 if hi == 256 else 0
                    L = (hi - lo) + vlo + vhi  # slots [0, L): real+virtual
                    buf = (pool or stage).tile([2 * D, 42, 258], BF16, tag=tag)
                    # zero-gap columns for dx-shift reads (both halves)
                    nc.vector.memset(buf[:, :, 0:1], 0.0)
                    nc.vector.memset(buf[:, :, 257:258], 0.0)
                    if vlo:
                        nc.vector.memset(buf[:, 0, :], 0.0)
                    if vhi:
                        nc.vector.memset(buf[:, L - 1, :], 0.0)
                    # top half (shifted dup) tail: slots [L-2, 42) never written
                    # by pair-dups; zero them so zero-weight K=128 taps stay finite
                    tail = max(L - 2, 0)
                    nc.vector.memset(buf[D:2 * D, tail:42, :], 0.0)
                    off = lo - vlo
                    return buf, off  # slot(row) = row - off

                t0, t0_off = mkstage("t0", t0_lo, t0_hi, pool=stage0)
                c1, c1_off = mkstage("c1", c1_lo, c1_hi)
                o3, o3_off = mkstage("o3", o3_lo, o3_hi)
                c3, c3_off = mkstage("c3", c3_lo, c3_hi)

                ybt = ybp.tile([D, 40, 256], BF16, tag="ybt")
                nc.sync.dma_start(
                    out=ybt[:, 0:t0_hi - t0_lo, :],
                    in_=yb[:, t0_lo * 256:t0_hi * 256])

                def dup(buf, q):
                    # top half slot t mirrors bottom slot t+1
                    a = max(q - 1, 0)
                    nc.gpsimd.tensor_copy(out=buf[D:2 * D, a:q + 1, :],
                                          in_=buf[0:D, a + 1:q + 2, :])

                # out2 = Wav @ y + bav  -> t0
                for pr in range(t0_lo, t0_hi, 2):
                    p = ps.tile([D, 512], F32)
                    nc.tensor.matmul(p, wavt_t, ybt[:, pr - t0_lo:pr - t0_lo + 2, :],
                                     start=True, stop=True)
                    nc.scalar.activation(out=t0[0:D, pr - t0_off:pr - t0_off + 2, 1:257],
                                         in_=p, func=AF.Identity, bias=bav_t, scale=1.0)
                    dup(t0, pr - t0_off)

                def conv(src, src_off, dst_lo, dst_hi, wtile):
                    wp, w2 = wtile
                    outs = []
                    for pr in range(dst_lo, dst_hi, 2):
                        p = ps.tile([D, 512], F32)
                        sl = pr - src_off
                        for kx in range(3):
                            # bottom: ky=0 rows (pr-1, pr); top: ky=1 rows (pr, pr+1)
                            nc.tensor.matmul(p, wp[:, kx * 64:(kx + 1) * 64],
                                             src[:, sl - 1:sl + 1, kx:kx + 256],
                                             start=(kx == 0), stop=False)
                        for kx in range(3):
                            # bottom: ky=2 rows (pr+1, pr+2); top: zero weights
                            nc.tensor.matmul(p, w2[:, kx * 64:(kx + 1) * 64],
                                             src[:, sl + 1:sl + 3, kx:kx + 256],
                                             start=False, stop=(kx == 2))
                        outs.append((pr, p))
                    return outs

                # conv1 + relu -> c1
                for pr, p in conv(t0, t0_off, c1_lo, c1_hi, wts[0]):
                    nc.scalar.activation(out=c1[0:D, pr - c1_off:pr - c1_off + 2, 1:257],
                                         in_=p, func=AF.Relu, bias=bts[0], scale=1.0)
                    dup(c1, pr - c1_off)
                # conv2 + bias + residual t0 -> o3
                for pr, p in conv(c1, c1_off, o3_lo, o3_hi, wts[1]):
                    nc.vector.scalar_tensor_tensor(
                        out=o3[0:D, pr - o3_off:pr - o3_off + 2, 1:257],
                        in0=p, scalar=bts[1],
                        in1=t0[0:D, pr - t0_off:pr - t0_off + 2, 1:257],
                        op0=OP.add, op1=OP.add)
                    dup(o3, pr - o3_off)
                # conv3 + relu -> c3
                for pr, p in conv(o3, o3_off, c3_lo, c3_hi, wts[2]):
                    nc.scalar.activation(out=c3[0:D, pr - c3_off:pr - c3_off + 2, 1:257],
                                         in_=p, func=AF.Relu, bias=bts[2], scale=1.0)
                    dup(c3, pr - c3_off)
                # conv4 + bias + residual o3 + y -> out
                for pr, p in conv(c3, c3_off, r0, r1, wts[3]):
                    stg = small.tile([D, 512], F32, tag="stg")
                    nc.vector.scalar_tensor_tensor(
                        out=stg, in0=p, scalar=bts[3],
                        in1=o3[0:D, pr - o3_off:pr - o3_off + 2, 1:257],
                        op0=OP.add, op1=OP.add)
                    yft = small.tile([D, 512], F32, tag="yft")
                    nc.sync.dma_start(out=yft, in_=yf[:, pr * 256:(pr + 2) * 256])
                    stg2 = small.tile([D, 512], F32, tag="stg2")
                    nc.vector.tensor_tensor(out=stg2, in0=stg, in1=yft, op=OP.add)
                    nc.sync.dma_start(out=out[:, pr * 256:(pr + 2) * 256], in_=stg2)
    nc.compile()
    return nc


def tile_pools(tc):
    import contextlib

    @contextlib.contextmanager
    def cm():
        with tc.tile_pool(name="consts", bufs=1) as consts, \
             tc.tile_pool(name="ybp", bufs=2) as ybp, \
             tc.tile_pool(name="stage0", bufs=2) as stage0, \
             tc.tile_pool(name="stage", bufs=1) as stage, \
             tc.tile_pool(name="small", bufs=4) as small, \
             tc.tile_pool(name="ps", bufs=8, space="PSUM") as ps:
            yield consts, ybp, stage0, stage, small, ps
    return cm()


_NC_CACHE = {}


def _get_ncs():
    if "a" not in _NC_CACHE:
        _NC_CACHE["a"] = _build_nc_a()
        _NC_CACHE["b"] = _build_nc_b()
    return _NC_CACHE["a"], _NC_CACHE["b"]


def _host_fold(sb, vw, vb):
    """S_big [128,128] -> (WavT bf16 [64,64], bav f32 [64,1])."""
    s = sb.astype(np.float64)
    S = s[0:64, 64:128]
    qss = np.diag(s[0:64, 0:64])
    kss = np.diag(s[64:128, 64:128])
    iq = 1.0 / np.maximum(np.sqrt(qss), 1e-12)
    ik = 1.0 / np.maximum(np.sqrt(kss), 1e-12)
    Ss = S * iq[:, None] * ik[None, :]
    A = np.zeros((64, 64), np.float64)
    for h in range(4):
        blk = Ss[16 * h:16 * h + 16, 16 * h:16 * h + 16]
        e = np.exp(blk - blk.max(axis=1, keepdims=True))
        A[16 * h:16 * h + 16, 16 * h:16 * h + 16] = e / e.sum(axis=1, keepdims=True)
    Wav = A @ vw.astype(np.float64)
    bav = A @ vb.astype(np.float64)
    return Wav.T.astype(np.float32).astype(BF), bav.astype(np.float32).reshape(64, 1)


def kernel(x, y, qw, qb, kw, kb, vw, vb,
           r1w1, r1b1, r1w2, r1b2, r2w1, r2b1, r2w2, r2b2, **_):
    x = np.asarray(x, np.float32)
    y = np.asarray(y, np.float32)
    qw, qb, kw, kb = (np.asarray(a, np.float32) for a in (qw, qb, kw, kb))
    vw, vb = np.asarray(vw, np.float32), np.asarray(vb, np.float32)
    r1w1, r1b1, r1w2, r1b2 = (np.asarray(a, np.float32) for a in (r1w1, r1b1, r1w2, r1b2))
    r2w1, r2b1, r2w2, r2b2 = (np.asarray(a, np.float32) for a in (r2w1, r2b1, r2w2, r2b2))
    nca, ncb = _get_ncs()

    ones = np.ones((1, HW), np.float32)
    wq_aug = np.concatenate([qw[:, :, 0, 0].T, qb[None, :]], axis=0).astype(BF)
    wk_aug = np.concatenate([kw[:, :, 0, 0].T, kb[None, :]], axis=0).astype(BF)

    in_maps_a = []
    for c in range(NCORES):
        xa = np.concatenate([x[c].reshape(D, HW), ones], axis=0).astype(BF)
        ya = np.concatenate([y[c].reshape(D, HW), ones], axis=0).astype(BF)
        in_maps_a.append({"xa": xa, "ya": ya, "wq": wq_aug, "wk": wk_aug})
    res_a = run_bass_kernel_spmd(nca, in_maps_a, core_ids=list(range(NCORES)))

    # host: softmax + fold attention into v-projection
    taps = {}
    for i, wc in ((1, r1w1), (2, r1w2), (3, r2w1), (4, r2w2)):
        wp = np.concatenate(
            [np.concatenate([wc[:, :, 0, kx].T, wc[:, :, 1, kx].T], axis=0)
             for kx in range(3)], axis=1)
        w2 = np.concatenate(
            [np.concatenate([wc[:, :, 2, kx].T, np.zeros((D, D), np.float32)], axis=0)
             for kx in range(3)], axis=1)
        taps[f"wp{i}"] = wp.astype(BF)
        taps[f"w2{i}"] = w2.astype(BF)
    biases = {"b1": r1b1, "b2": r1b2, "b3": r2b1, "b4": r2b2}

    in_maps_b = []
    for c in range(NCORES):
        wavt, bav = _host_fold(res_a.results[c]["sbig"], vw[:, :, 0, 0], vb)
        m = {"yb": y[c].reshape(D, HW).astype(BF),
             "yf": np.ascontiguousarray(y[c].reshape(D, HW)),
             "wavt": wavt, "bav": bav}
        for nm, v in taps.items():
            m[nm] = v
        for nm, v in biases.items():
            m[nm] = np.ascontiguousarray(v.astype(np.float32).reshape(D, 1))
        in_maps_b.append(m)
    res_b = run_bass_kernel_spmd(ncb, in_maps_b, core_ids=list(range(NCORES)))

    return np.stack([res_b.results[c]["out"].reshape(D, H, W_IMG)
                     for c in range(NCORES)]).astype(np.float32)


if __name__ == "__main__":
    rng = np.random.default_rng(0)
    ins = {
        "x": rng.standard_normal((8, D, H, W_IMG)).astype(np.float32),
        "y": rng.standard_normal((8, D, H, W_IMG)).astype(np.float32),
        "qw": (rng.standard_normal((D, D, 1, 1)) / 8).astype(np.float32),
        "qb": (rng.standard_normal(D) / 8).astype(np.float32),
        "kw": (rng.standard_normal((D, D, 1, 1)) / 8).astype(np.float32),
        "kb": (rng.standard_normal(D) / 8).astype(np.float32),
        "vw": (rng.standard_normal((D, D, 1, 1)) / 8).astype(np.float32),
        "vb": (rng.standard_normal(D) / 8).astype(np.float32),
    }
    for i in (1, 2):
        for j in (1, 2):
            ins[f"r{i}w{j}"] = (rng.standard_normal((D, D, 3, 3)) / 24).astype(np.float32)
            ins[f"r{i}b{j}"] = (rng.standard_normal(D) / 24).astype(np.float32)
    o = kernel(**ins)
    print("kernel ran, out shape", o.shape, "std", o.std())



# revision 12
# speedup vs baseline: 2.3192x; 2.3192x over previous
"""Trainium2 Bass kernel for channel-attention + 2 residual conv blocks.

Data-parallel over batch (8 cores, 1 batch element each). Two SPMD launches:
  A) raw channel Gram G = [x;y]^T-pixel-contraction ([128,128]) via 512
     accumulating matmuls on pixel-major data (host pre-transposes); no
     per-tile PSUM->SBUF copies at all.
  B) fused attention-apply + 4 3x3 convs. Convs run in fp8-e4m3 DoubleRow
     matmuls (2 k-tiles replace the row-shifted duplicate half entirely),
     packed 4 output rows per matmul group (M=128=2rows x 64ch,
     N=512=2 slot-pairs x 256 cols). Residual paths stay exact via bf16
     identity/Wav injection matmuls into PSUM; out rows DMA straight from
     PSUM. Host does only the O(64^2) softmax/fold algebra between launches.
"""
import sys, os
for p in ('/opt/trn_rl_repo', os.path.expanduser('~/.axon_site/_ro/trn_rl_repo')):
    if os.path.isdir(p) and p not in sys.path:
        sys.path.insert(0, p)

import numpy as np
import ml_dtypes
import concourse.bass as bass
import concourse.bacc as bacc
import concourse.tile as tile
from concourse import mybir
from concourse.bass_utils import run_bass_kernel_spmd

dt = mybir.dt
F32, BF16, FP8 = dt.float32, dt.bfloat16, dt.float8e4
BF = ml_dtypes.bfloat16
E4 = ml_dtypes.float8_e4m3
AF = mybir.ActivationFunctionType
OP = mybir.AluOpType
DR = mybir.MatmulPerfMode.DoubleRow

D = 64
HW = 65536
H = W_IMG = 256
NCORES = 8
R = 32  # output rows per block in pass B


def _build_nc_a():
    nc = bacc.Bacc("TRN2", target_bir_lowering=False, debug=False)
    za = nc.dram_tensor("za", [32, 128, 2048], BF16, kind="ExternalInput").ap()
    gout = nc.dram_tensor("gout", [128, 128], F32, kind="ExternalOutput").ap()
    with tile.TileContext(nc) as tc:
        with tc.tile_pool(name="io", bufs=3) as io, \
             tc.tile_pool(name="work", bufs=1) as work, \
             tc.tile_pool(name="gps", bufs=1, space="PSUM") as gps:
            gp = gps.tile([128, 128], F32)
            for t in range(32):
                zt = io.tile([128, 2048], BF16, tag="zt")
                nc.sync.dma_start(out=zt, in_=za[t])
                for j in range(16):
                    s = zt[:, j * 128:(j + 1) * 128]
                    nc.tensor.matmul(gp, s, s,
                                     start=(t == 0 and j == 0),
                                     stop=(t == 31 and j == 15),
                                     skip_group_check=True)
            gs = work.tile([128, 128], F32)
            nc.vector.tensor_copy(out=gs, in_=gp)
            nc.sync.dma_start(out=gout, in_=gs)
    nc.compile()
    return nc


def _emit_groups(lo, hi):
    """4-row groups (+2-row remainder; odd counts overlap by one row)."""
    out, g, n = [], lo, hi - lo
    while n >= 4:
        out.append((g, 4)); g += 4; n -= 4
    if n == 3:
        out.append((hi - 4, 4))
    elif n == 2:
        out.append((g, 2))
    elif n == 1:
        out.append((hi - 2, 2))
    return out


def _build_nc_b():
    nc = bacc.Bacc("TRN2", target_bir_lowering=False, debug=False)
    yb = nc.dram_tensor("yb", [D, H, W_IMG], BF16, kind="ExternalInput").ap()
    wavt = nc.dram_tensor("wavt", [D, D], BF16, kind="ExternalInput").ap()
    ii_d = nc.dram_tensor("ii", [2 * D, D], BF16, kind="ExternalInput").ap()
    # fp8 DoubleRow weights: per conv, per kx, (a|b) variant [64, 2, 128]
    wdr_d = {}
    for c in range(1, 5):
        for kx in range(3):
            for v in 'ab':
                nm = f"w{c}{kx}{v}"
                wdr_d[nm] = nc.dram_tensor(nm, [D, 2, 2 * D], FP8,
                                           kind="ExternalInput").ap()
    bias_d = {nm: nc.dram_tensor(nm, [D, 1], F32, kind="ExternalInput").ap()
              for nm in ('bt0', 'bc1', 'bo3b', 'bn4', 'bc3')}
    out_d = nc.dram_tensor("out", [D, H, W_IMG], F32, kind="ExternalOutput").ap()

    with tile.TileContext(nc) as tc:
        with tc.tile_pool(name="consts", bufs=1) as consts, \
             tc.tile_pool(name="stg", bufs=1) as stg, \
             tc.tile_pool(name="oyp", bufs=2) as oyp, \
             tc.tile_pool(name="outs", bufs=4) as outs, \
             tc.tile_pool(name="ps", bufs=4, space="PSUM") as ps, \
             tc.tile_pool(name="ps2", bufs=2, space="PSUM") as ps2:
            wavt_t = consts.tile([D, D], BF16)
            ii_t = consts.tile([2 * D, D], BF16)
            nc.sync.dma_start(out=wavt_t, in_=wavt)
            nc.sync.dma_start(out=ii_t, in_=ii_d)
            wdr = {}
            for nm, d in wdr_d.items():
                t = consts.tile([D, 2, 2 * D], FP8, tag=nm)
                nc.sync.dma_start(out=t, in_=d)
                wdr[nm] = t
            bias = {}
            for nm, d in bias_d.items():
                t = consts.tile([D, 1], F32, tag=nm)
                nc.sync.dma_start(out=t, in_=d)
                bias[nm] = t

            # persistent fp8 stage buffers; col 0/257 stay zero forever
            t0 = stg.tile([D, 258, 258], FP8)   # slot = row + 1 (full image)
            c1 = stg.tile([D, 40, 258], FP8)
            o3f = stg.tile([D, 38, 258], FP8)
            c3 = stg.tile([D, 36, 258], FP8)
            for t in (t0, c1, o3f, c3):
                nc.vector.memset(t[:, :, 0:1], 0.0)
                nc.vector.memset(t[:, :, 257:258], 0.0)
            nc.vector.memset(t0[:, 0:1, :], 0.0)     # virtual row -1
            nc.vector.memset(t0[:, 257:258, :], 0.0)  # virtual row 256

            # stage-write engine rotation: psum -> stage with bias (+relu)
            rot = [0]

            def wr(out_ap, in_ap, b, relu):
                # PSUM readers: alternate ACT / DVE (GPSIMD cannot read PSUM)
                e = rot[0] % 2
                rot[0] += 1
                if e == 0:
                    nc.scalar.activation(out=out_ap, in_=in_ap,
                                         func=(AF.Relu if relu else AF.Identity),
                                         bias=b, scale=1.0)
                elif relu:
                    nc.vector.tensor_scalar(out=out_ap, in0=in_ap, scalar1=b,
                                            scalar2=0.0, op0=OP.add, op1=OP.max)
                else:
                    nc.vector.tensor_scalar_add(out=out_ap, in0=in_ap, scalar1=b)

            # --- block loop -------------------------------------------------
            t0_done = 0  # t0 rows produced so far
            for blk in range(8):
                r0, r1 = blk * R, (blk + 1) * R
                oy = oyp.tile([2 * D, 44, W_IMG], BF16, tag="oy")

                def oslot(row):
                    return row - (r0 - 4)

                ylo, yhi = max(r0 - 4, 0), min(r1 + 4, 256)
                nc.sync.dma_start(out=oy[0:D, oslot(ylo):oslot(yhi), :],
                                  in_=yb[:, ylo:yhi, :])

                # stage ranges (produced rows)
                c1lo, c1hi = max(r0 - 3, 0), min(r1 + 3, 256)
                o3lo, o3hi = max(r0 - 2, 0), min(r1 + 2, 256)
                c3lo, c3hi = max(r0 - 1, 0), min(r1 + 1, 256)

                def s_c1(row): return row - c1lo + 1
                def s_o3(row): return row - o3lo + 1
                def s_c3(row): return row - c3lo + 1
                def s_t0(row): return row + 1

                # virtual zero rows at image edges (persistent tiles: emit
                # only when the slot is actually consumed as a virtual row)
                if blk == 0:
                    for t in (c1, o3f, c3):
                        nc.vector.memset(t[:, 0:1, :], 0.0)
                if blk == 7:
                    nc.vector.memset(c1[:, s_c1(256):s_c1(256) + 1, :], 0.0)
                    nc.vector.memset(o3f[:, s_o3(256):s_o3(256) + 1, :], 0.0)
                    nc.vector.memset(c3[:, s_c3(256):s_c3(256) + 1, :], 0.0)

                # ---- t0 (= Wav y + bav), full-image persistent, 4-row steps
                t0_hi = min(r1 + 4, 256)
                for g in range(t0_done, t0_hi, 4):
                    p2 = ps2.tile([D, 1024], F32, tag="p2")
                    for h in range(2):
                        nc.tensor.matmul(
                            p2[:, h * 512:(h + 1) * 512], wavt_t,
                            oy[0:D, oslot(g + 2 * h):oslot(g + 2 * h) + 2, :],
                            start=True, stop=True, skip_group_check=True)
                    wr(t0[:, s_t0(g):s_t0(g) + 4, 1:257], p2, bias['bt0'], False)
                t0_done = t0_hi

                # ---- conv1: t0 -> c1 (relu, b1)
                for g, sz in _emit_groups(c1lo, c1hi):
                    ssz = sz // 2
                    p = ps.tile([2 * D, 128 * sz], F32, tag="cv")
                    last = None
                    for i, sig in enumerate((g - 1, g + 1)):
                        sl = s_t0(sig)
                        for kx in range(3):
                            mv = t0[:, sl:sl + 2 * ssz, kx:kx + 256]
                            mv = mv.rearrange("p (s t) c -> p t s c", t=2)
                            nc.tensor.matmul(p, wdr[f"w1{kx}{'ab'[i]}"], mv,
                                             start=(i == 0 and kx == 0),
                                             stop=(i == 1 and kx == 2),
                                             perf_mode=DR, skip_group_check=True)
                    for rho in range(2):
                        dst = c1[:, s_c1(g + rho):s_c1(g + rho) + 2 * ssz:2, 1:257]
                        wr(dst, p[rho * D:(rho + 1) * D, :], bias['bc1'], True)

                # ---- conv2 + Wav-inject: c1 -> o3 (fp8 + bf16-in-oy)
                for g, sz in _emit_groups(o3lo, o3hi):
                    ssz = sz // 2
                    p = ps.tile([2 * D, 128 * sz], F32, tag="cv")
                    for i, sig in enumerate((g - 1, g + 1)):
                        sl = s_c1(sig)
                        for kx in range(3):
                            mv = c1[:, sl:sl + 2 * ssz, kx:kx + 256]
                            mv = mv.rearrange("p (s t) c -> p t s c", t=2)
                            nc.tensor.matmul(p, wdr[f"w2{kx}{'ab'[i]}"], mv,
                                             start=(i == 0 and kx == 0),
                                             stop=False,
                                             perf_mode=DR, skip_group_check=True)
                    for rho in range(2):
                        nc.tensor.matmul(
                            p[rho * D:(rho + 1) * D, :], wavt_t,
                            oy[0:D, oslot(g + rho):oslot(g + rho) + 2 * ssz:2, :],
                            start=False, stop=(rho == 1), skip_group_check=True)
                    for rho in range(2):
                        psl = p[rho * D:(rho + 1) * D, :]
                        dstb = oy[D:2 * D, oslot(g + rho):oslot(g + rho) + 2 * ssz:2, :]
                        wr(dstb, psl, bias['bo3b'], False)
                        # fp8 copy for conv3 input: GPSIMD from the bf16 o3
                        # (o3_bf16 carries +b4; subtract it again here)
                        dst = o3f[:, s_o3(g + rho):s_o3(g + rho) + 2 * ssz:2, 1:257]
                        nc.gpsimd.tensor_scalar_add(out=dst, in0=dstb,
                                                    scalar1=bias['bn4'])

                # ---- conv3: o3f -> c3 (relu, b3)
                for g, sz in _emit_groups(c3lo, c3hi):
                    ssz = sz // 2
                    p = ps.tile([2 * D, 128 * sz], F32, tag="cv")
                    for i, sig in enumerate((g - 1, g + 1)):
                        sl = s_o3(sig)
                        for kx in range(3):
                            mv = o3f[:, sl:sl + 2 * ssz, kx:kx + 256]
                            mv = mv.rearrange("p (s t) c -> p t s c", t=2)
                            nc.tensor.matmul(p, wdr[f"w3{kx}{'ab'[i]}"], mv,
                                             start=(i == 0 and kx == 0),
                                             stop=(i == 1 and kx == 2),
                                             perf_mode=DR, skip_group_check=True)
                    for rho in range(2):
                        dst = c3[:, s_c3(g + rho):s_c3(g + rho) + 2 * ssz:2, 1:257]
                        wr(dst, p[rho * D:(rho + 1) * D, :], bias['bc3'], True)

                # ---- conv4 + (o3+b4+y)-inject: c3 -> out (DMA from PSUM)
                for g in range(r0, r1, 4):
                    p = ps.tile([2 * D, 512], F32, tag="cv")
                    for i, sig in enumerate((g - 1, g + 1)):
                        sl = s_c3(sig)
                        for kx in range(3):
                            mv = c3[:, sl:sl + 4, kx:kx + 256]
                            mv = mv.rearrange("p (s t) c -> p t s c", t=2)
                            nc.tensor.matmul(p, wdr[f"w4{kx}{'ab'[i]}"], mv,
                                             start=(i == 0 and kx == 0),
                                             stop=False,
                                             perf_mode=DR, skip_group_check=True)
                    for rho in range(2):
                        nc.tensor.matmul(
                            p[rho * D:(rho + 1) * D, :], ii_t,
                            oy[:, oslot(g + rho):oslot(g + rho) + 4:2, :],
                            start=False, stop=(rho == 1), skip_group_check=True)
                    for rho in range(2):
                        so = outs.tile([D, 512], F32, tag="so")
                        e = rot[0] % 2
                        rot[0] += 1
                        if e == 0:
                            nc.scalar.activation(out=so, in_=p[rho * D:(rho + 1) * D, :],
                                                 func=AF.Copy, bias=0.0, scale=1.0)
                        else:
                            nc.vector.tensor_copy(out=so, in_=p[rho * D:(rho + 1) * D, :])
                        nc.sync.dma_start(
                            out=out_d[:, g + rho:g + rho + 3:2, :], in_=so)
    nc.compile()
    return nc


_NC_CACHE = {}


def _get_ncs():
    if "a" not in _NC_CACHE:
        _NC_CACHE["a"] = _build_nc_a()
        _NC_CACHE["b"] = _build_nc_b()
    return _NC_CACHE["a"], _NC_CACHE["b"]


def _host_fold(G, Sx, Sy, Wq, bq, Wk, bk, Vw, vb):
    """Raw Gram [128,128] + channel sums -> (Wav [64,64], bav [64]) in f64."""
    G = G.astype(np.float64)
    Gxx, Gxy, Gyy = G[:D, :D], G[:D, D:], G[D:, D:]
    n = float(HW)
    QK = (Wq @ Gxy @ Wk.T + np.outer(Wq @ Sx, bk)
          + np.outer(bq, Wk @ Sy) + n * np.outer(bq, bk))
    qq = np.einsum('ij,jk,ik->i', Wq, Gxx, Wq) + 2 * bq * (Wq @ Sx) + n * bq * bq
    kk = np.einsum('ij,jk,ik->i', Wk, Gyy, Wk) + 2 * bk * (Wk @ Sy) + n * bk * bk
    St = QK / np.maximum(np.sqrt(qq), 1e-12)[:, None] \
            / np.maximum(np.sqrt(kk), 1e-12)[None, :]
    A = np.zeros((D, D))
    for h in range(4):
        blk = St[16 * h:16 * h + 16, 16 * h:16 * h + 16]
        e = np.exp(blk - blk.max(axis=1, keepdims=True))
        A[16 * h:16 * h + 16, 16 * h:16 * h + 16] = e / e.sum(axis=1, keepdims=True)
    return A @ Vw, A @ vb


def _prep_dr_weights(w):
    """w [64o, 64i, 3, 3] f32 -> dict kx -> (Wa, Wb) [64, 2, 128] e4m3."""
    out = {}
    for kx in range(3):
        Wa = np.zeros((D, 2, 2 * D), np.float32)
        Wb = np.zeros((D, 2, 2 * D), np.float32)
        wt = w[:, :, :, kx]  # [o, i, ky]
        Wa[:, 0, 0:D] = wt[:, :, 0].T
        Wa[:, 1, 0:D] = wt[:, :, 1].T
        Wa[:, 1, D:2 * D] = wt[:, :, 0].T
        Wb[:, 0, 0:D] = wt[:, :, 2].T
        Wb[:, 0, D:2 * D] = wt[:, :, 1].T
        Wb[:, 1, D:2 * D] = wt[:, :, 2].T
        out[kx] = (Wa.astype(E4), Wb.astype(E4))
    return out


def kernel(x, y, qw, qb, kw, kb, vw, vb,
           r1w1, r1b1, r1w2, r1b2, r2w1, r2b1, r2w2, r2b2, **_):
    x = np.asarray(x, np.float32)
    y = np.asarray(y, np.float32)
    qw, qb, kw, kb = (np.asarray(a, np.float32) for a in (qw, qb, kw, kb))
    vw, vb = np.asarray(vw, np.float32), np.asarray(vb, np.float32)
    r1w1, r1b1, r1w2, r1b2 = (np.asarray(a, np.float32) for a in (r1w1, r1b1, r1w2, r1b2))
    r2w1, r2b1, r2w2, r2b2 = (np.asarray(a, np.float32) for a in (r2w1, r2b1, r2w2, r2b2))
    nca, ncb = _get_ncs()

    # ---- pass A: pixel-major Gram
    in_maps_a = []
    xs_l, ys_l = [], []
    for c in range(NCORES):
        xc = x[c].reshape(D, HW)
        yc = y[c].reshape(D, HW)
        xs_l.append(xc.sum(axis=1, dtype=np.float64))
        ys_l.append(yc.sum(axis=1, dtype=np.float64))
        Z = np.empty((HW, 2 * D), np.float32)
        Z[:, :D] = xc.T
        Z[:, D:] = yc.T
        za = Z.reshape(32, 16, 128, 128).transpose(0, 2, 1, 3) \
              .reshape(32, 128, 2048).astype(BF)
        in_maps_a.append({"za": np.ascontiguousarray(za)})
    res_a = run_bass_kernel_spmd(nca, in_maps_a, core_ids=list(range(NCORES)))

    # ---- host fold + pass-B constants
    Wq, Wk, Vw = qw[:, :, 0, 0].astype(np.float64), kw[:, :, 0, 0].astype(np.float64), \
        vw[:, :, 0, 0].astype(np.float64)
    bq64, bk64, vb64 = qb.astype(np.float64), kb.astype(np.float64), vb.astype(np.float64)
    wdr_np = {}
    for ci, w in ((1, r1w1), (2, r1w2), (3, r2w1), (4, r2w2)):
        d = _prep_dr_weights(w)
        for kx in range(3):
            wdr_np[f"w{ci}{kx}a"] = d[kx][0]
            wdr_np[f"w{ci}{kx}b"] = d[kx][1]
    ii = np.concatenate([np.eye(D, dtype=np.float32)] * 2, axis=0).astype(BF)

    in_maps_b = []
    for c in range(NCORES):
        Wav, bav = _host_fold(res_a.results[c]["gout"], xs_l[c], ys_l[c],
                              Wq, bq64, Wk, bk64, Vw, vb64)
        m = {"yb": np.ascontiguousarray(y[c].reshape(D, H, W_IMG).astype(BF)),
             "wavt": np.ascontiguousarray(Wav.T.astype(np.float32).astype(BF)),
             "ii": ii,
             "bt0": bav.astype(np.float32).reshape(D, 1),
             "bc1": r1b1.reshape(D, 1),
             "bo3b": (bav + r1b2 + r2b2).astype(np.float32).reshape(D, 1),
             "bn4": (-r2b2).astype(np.float32).reshape(D, 1),
             "bc3": r2b1.reshape(D, 1)}
        m.update(wdr_np)
        in_maps_b.append({k: np.ascontiguousarray(v) for k, v in m.items()})
    res_b = run_bass_kernel_spmd(ncb, in_maps_b, core_ids=list(range(NCORES)))

    return np.stack([res_b.results[c]["out"].reshape(D, H, W_IMG)
                     for c in range(NCORES)]).astype(np.float32)


if __name__ == "__main__":
    rng = np.random.default_rng(0)
    ins = {
        "x": rng.standard_normal((8, D, H, W_IMG)).astype(np.float32),
        "y": rng.standard_normal((8, D, H, W_IMG)).astype(np.float32),
        "qw": (rng.standard_normal((D, D, 1, 1)) / 8).astype(np.float32),
        "qb": (rng.standard_normal(D) / 8).astype(np.float32),
        "kw": (rng.standard_normal((D, D, 1, 1)) / 8).astype(np.float32),
        "kb": (rng.standard_normal(D) / 8).astype(np.float32),
        "vw": (rng.standard_normal((D, D, 1, 1)) / 8).astype(np.float32),
        "vb": (rng.standard_normal(D) / 8).astype(np.float32),
    }
    for i in (1, 2):
        for j in (1, 2):
            ins[f"r{i}w{j}"] = (rng.standard_normal((D, D, 3, 3)) / 24).astype(np.float32)
            ins[f"r{i}b{j}"] = (rng.standard_normal(D) / 24).astype(np.float32)
    o = kernel(**ins)
    print("kernel ran, out shape", o.shape, "std", o.std())


# revision 15
# speedup vs baseline: 2.4924x; 1.0747x over previous
"""Trainium2 Bass kernel for channel-attention + 2 residual conv blocks.

Data-parallel over batch (8 cores, 1 batch element each). Two SPMD launches:
  A) raw channel Gram G = [x;y]^T-pixel-contraction ([128,128]) via 512
     accumulating matmuls on pixel-major data (host pre-transposes); no
     per-tile PSUM->SBUF copies at all.
  B) fused attention-apply + 4 3x3 convs. Convs run in fp8-e4m3 DoubleRow
     matmuls (2 k-tiles replace the row-shifted duplicate half entirely),
     packed 4 output rows per matmul group (M=128=2rows x 64ch,
     N=512=2 slot-pairs x 256 cols). Residual paths stay exact via bf16
     identity/Wav injection matmuls into PSUM; out rows DMA straight from
     PSUM. Host does only the O(64^2) softmax/fold algebra between launches.
"""
import sys, os
for p in ('/opt/trn_rl_repo', os.path.expanduser('~/.axon_site/_ro/trn_rl_repo')):
    if os.path.isdir(p) and p not in sys.path:
        sys.path.insert(0, p)

import numpy as np
import ml_dtypes
import concourse.bass as bass
import concourse.bacc as bacc
import concourse.tile as tile
from concourse import mybir
from concourse.bass_utils import run_bass_kernel_spmd

dt = mybir.dt
F32, BF16, FP8 = dt.float32, dt.bfloat16, dt.float8e4
BF = ml_dtypes.bfloat16
E4 = ml_dtypes.float8_e4m3
AF = mybir.ActivationFunctionType
OP = mybir.AluOpType
DR = mybir.MatmulPerfMode.DoubleRow

D = 64
HW = 65536
H = W_IMG = 256
NCORES = 8
R = 32  # output rows per block in pass B


def _build_nc_a():
    nc = bacc.Bacc("TRN2", target_bir_lowering=False, debug=False)
    za = nc.dram_tensor("za", [32, 128, 2048], BF16, kind="ExternalInput").ap()
    gout = nc.dram_tensor("gout", [128, 128], F32, kind="ExternalOutput").ap()
    with tile.TileContext(nc) as tc:
        with tc.tile_pool(name="io", bufs=3) as io, \
             tc.tile_pool(name="work", bufs=1) as work, \
             tc.tile_pool(name="gps", bufs=1, space="PSUM") as gps:
            gp = gps.tile([128, 128], F32)
            for t in range(32):
                zt = io.tile([128, 2048], BF16, tag="zt")
                nc.sync.dma_start(out=zt, in_=za[t])
                for j in range(16):
                    s = zt[:, j * 128:(j + 1) * 128]
                    nc.tensor.matmul(gp, s, s,
                                     start=(t == 0 and j == 0),
                                     stop=(t == 31 and j == 15),
                                     skip_group_check=True)
            gs = work.tile([128, 128], F32)
            nc.vector.tensor_copy(out=gs, in_=gp)
            nc.sync.dma_start(out=gout, in_=gs)
    nc.compile()
    return nc


def _emit_groups(lo, hi):
    """4-row groups (+2-row remainder; odd counts overlap by one row)."""
    out, g, n = [], lo, hi - lo
    while n >= 4:
        out.append((g, 4)); g += 4; n -= 4
    if n == 3:
        out.append((hi - 4, 4))
    elif n == 2:
        out.append((g, 2))
    elif n == 1:
        out.append((hi - 2, 2))
    return out


def _build_nc_b():
    nc = bacc.Bacc("TRN2", target_bir_lowering=False, debug=False)
    yb = nc.dram_tensor("yb", [D, H, W_IMG], BF16, kind="ExternalInput").ap()
    wavt = nc.dram_tensor("wavt", [D, D], BF16, kind="ExternalInput").ap()
    ii_d = nc.dram_tensor("ii", [2 * D, D], BF16, kind="ExternalInput").ap()
    # fp8 DoubleRow weights: per conv, per kx, (a|b) variant [64, 2, 128]
    wdr_d = {}
    for c in range(1, 5):
        for kx in range(3):
            for v in 'ab':
                nm = f"w{c}{kx}{v}"
                wdr_d[nm] = nc.dram_tensor(nm, [D, 2, 2 * D], FP8,
                                           kind="ExternalInput").ap()
    bias_d = {nm: nc.dram_tensor(nm, [D, 1], F32, kind="ExternalInput").ap()
              for nm in ('bt0', 'bc1', 'bo3b', 'bn4', 'bc3')}
    out_d = nc.dram_tensor("out", [D, H, W_IMG], F32, kind="ExternalOutput").ap()

    with tile.TileContext(nc) as tc:
        with tc.tile_pool(name="consts", bufs=1) as consts, \
             tc.tile_pool(name="stg", bufs=1) as stg, \
             tc.tile_pool(name="oyp", bufs=2) as oyp, \
             tc.tile_pool(name="outs", bufs=4) as outs, \
             tc.tile_pool(name="ps", bufs=4, space="PSUM") as ps, \
             tc.tile_pool(name="ps2", bufs=2, space="PSUM") as ps2:
            wavt_t = consts.tile([D, D], BF16)
            ii_t = consts.tile([2 * D, D], BF16)
            nc.sync.dma_start(out=wavt_t, in_=wavt)
            nc.sync.dma_start(out=ii_t, in_=ii_d)
            wdr = {}
            for nm, d in wdr_d.items():
                t = consts.tile([D, 2, 2 * D], FP8, tag=nm)
                nc.sync.dma_start(out=t, in_=d)
                wdr[nm] = t
            bias = {}
            for nm, d in bias_d.items():
                t = consts.tile([D, 1], F32, tag=nm)
                nc.sync.dma_start(out=t, in_=d)
                bias[nm] = t

            # persistent fp8 stage buffers; col 0/257 stay zero forever
            t0 = stg.tile([D, 258, 258], FP8)   # slot = row + 1 (full image)
            c1 = stg.tile([D, 40, 258], FP8)
            o3f = stg.tile([D, 38, 258], FP8)
            c3 = stg.tile([D, 36, 258], FP8)
            for t in (t0, c1, o3f, c3):
                nc.vector.memset(t[:, :, 0:1], 0.0)
                nc.vector.memset(t[:, :, 257:258], 0.0)
            nc.vector.memset(t0[:, 0:1, :], 0.0)     # virtual row -1
            nc.vector.memset(t0[:, 257:258, :], 0.0)  # virtual row 256

            # stage-write engine balance: psum -> stage with bias (+relu).
            # GPSIMD cannot read PSUM, so these go to ACT/DVE, weighted by
            # modeled per-op cost (ACT 0.833ns/el +143, DVE 1.042ns/el +125).
            acc = [0.0, 0.0]

            def wr(out_ap, in_ap, b, relu):
                n = out_ap.free_size()
                ca, cd = n * 0.833 + 143.0, n * 1.042 + 125.0
                if acc[0] + ca <= acc[1] + cd:
                    acc[0] += ca
                    nc.scalar.activation(out=out_ap, in_=in_ap,
                                         func=(AF.Relu if relu else AF.Identity),
                                         bias=b, scale=1.0)
                elif relu:
                    acc[1] += cd
                    nc.vector.tensor_scalar(out=out_ap, in0=in_ap, scalar1=b,
                                            scalar2=0.0, op0=OP.add, op1=OP.max)
                else:
                    acc[1] += cd
                    nc.vector.tensor_scalar_add(out=out_ap, in0=in_ap, scalar1=b)

            # --- block loop -------------------------------------------------
            t0_done = 0  # t0 rows produced so far
            for blk in range(8):
                r0, r1 = blk * R, (blk + 1) * R
                oy = oyp.tile([2 * D, 44, W_IMG], BF16, tag="oy")

                def oslot(row):
                    return row - (r0 - 4)

                ylo, yhi = max(r0 - 4, 0), min(r1 + 4, 256)
                nc.sync.dma_start(out=oy[0:D, oslot(ylo):oslot(yhi), :],
                                  in_=yb[:, ylo:yhi, :])

                # stage ranges (produced rows)
                c1lo, c1hi = max(r0 - 3, 0), min(r1 + 3, 256)
                o3lo, o3hi = max(r0 - 2, 0), min(r1 + 2, 256)
                c3lo, c3hi = max(r0 - 1, 0), min(r1 + 1, 256)

                def s_c1(row): return row - c1lo + 1
                def s_o3(row): return row - o3lo + 1
                def s_c3(row): return row - c3lo + 1
                def s_t0(row): return row + 1

                # virtual zero rows at image edges (persistent tiles: emit
                # only when the slot is actually consumed as a virtual row)
                if blk == 0:
                    for t in (c1, o3f, c3):
                        nc.vector.memset(t[:, 0:1, :], 0.0)
                if blk == 7:
                    nc.vector.memset(c1[:, s_c1(256):s_c1(256) + 1, :], 0.0)
                    nc.vector.memset(o3f[:, s_o3(256):s_o3(256) + 1, :], 0.0)
                    nc.vector.memset(c3[:, s_c3(256):s_c3(256) + 1, :], 0.0)

                # ---- t0 (= Wav y + bav), full-image persistent, 4-row steps
                t0_hi = min(r1 + 4, 256)
                for g in range(t0_done, t0_hi, 4):
                    p2 = ps2.tile([D, 1024], F32, tag="p2")
                    for h in range(2):
                        nc.tensor.matmul(
                            p2[:, h * 512:(h + 1) * 512], wavt_t,
                            oy[0:D, oslot(g + 2 * h):oslot(g + 2 * h) + 2, :],
                            start=True, stop=True, skip_group_check=True)
                    wr(t0[:, s_t0(g):s_t0(g) + 4, 1:257], p2, bias['bt0'], False)
                t0_done = t0_hi

                # ---- conv1: t0 -> c1 (relu, b1)
                for g, sz in _emit_groups(c1lo, c1hi):
                    ssz = sz // 2
                    p = ps.tile([2 * D, 128 * sz], F32, tag="cv")
                    last = None
                    for i, sig in enumerate((g - 1, g + 1)):
                        sl = s_t0(sig)
                        for kx in range(3):
                            mv = t0[:, sl:sl + 2 * ssz, kx:kx + 256]
                            mv = mv.rearrange("p (s t) c -> p t s c", t=2)
                            nc.tensor.matmul(p, wdr[f"w1{kx}{'ab'[i]}"], mv,
                                             start=(i == 0 and kx == 0),
                                             stop=(i == 1 and kx == 2),
                                             perf_mode=DR, skip_group_check=True)
                    for rho in range(2):
                        dst = c1[:, s_c1(g + rho):s_c1(g + rho) + 2 * ssz:2, 1:257]
                        wr(dst, p[rho * D:(rho + 1) * D, :], bias['bc1'], True)

                # ---- conv2 + Wav-inject: c1 -> o3 (fp8 + bf16-in-oy)
                for g, sz in _emit_groups(o3lo, o3hi):
                    ssz = sz // 2
                    p = ps.tile([2 * D, 128 * sz], F32, tag="cv")
                    for i, sig in enumerate((g - 1, g + 1)):
                        sl = s_c1(sig)
                        for kx in range(3):
                            mv = c1[:, sl:sl + 2 * ssz, kx:kx + 256]
                            mv = mv.rearrange("p (s t) c -> p t s c", t=2)
                            nc.tensor.matmul(p, wdr[f"w2{kx}{'ab'[i]}"], mv,
                                             start=(i == 0 and kx == 0),
                                             stop=False,
                                             perf_mode=DR, skip_group_check=True)
                    for rho in range(2):
                        nc.tensor.matmul(
                            p[rho * D:(rho + 1) * D, :], wavt_t,
                            oy[0:D, oslot(g + rho):oslot(g + rho) + 2 * ssz:2, :],
                            start=False, stop=(rho == 1), skip_group_check=True)
                    for rho in range(2):
                        psl = p[rho * D:(rho + 1) * D, :]
                        dstb = oy[D:2 * D, oslot(g + rho):oslot(g + rho) + 2 * ssz:2, :]
                        wr(dstb, psl, bias['bo3b'], False)
                        # fp8 copy for conv3 input: GPSIMD from the bf16 o3
                        # (o3_bf16 carries +b4; subtract it again here)
                        dst = o3f[:, s_o3(g + rho):s_o3(g + rho) + 2 * ssz:2, 1:257]
                        nc.gpsimd.tensor_scalar_add(out=dst, in0=dstb,
                                                    scalar1=bias['bn4'])

                # ---- conv3: o3f -> c3 (relu, b3)
                for g, sz in _emit_groups(c3lo, c3hi):
                    ssz = sz // 2
                    p = ps.tile([2 * D, 128 * sz], F32, tag="cv")
                    for i, sig in enumerate((g - 1, g + 1)):
                        sl = s_o3(sig)
                        for kx in range(3):
                            mv = o3f[:, sl:sl + 2 * ssz, kx:kx + 256]
                            mv = mv.rearrange("p (s t) c -> p t s c", t=2)
                            nc.tensor.matmul(p, wdr[f"w3{kx}{'ab'[i]}"], mv,
                                             start=(i == 0 and kx == 0),
                                             stop=(i == 1 and kx == 2),
                                             perf_mode=DR, skip_group_check=True)
                    for rho in range(2):
                        dst = c3[:, s_c3(g + rho):s_c3(g + rho) + 2 * ssz:2, 1:257]
                        wr(dst, p[rho * D:(rho + 1) * D, :], bias['bc3'], True)

                # ---- conv4 + (o3+b4+y)-inject: c3 -> out (DMA from PSUM)
                for g in range(r0, r1, 4):
                    p = ps.tile([2 * D, 512], F32, tag="cv")
                    for i, sig in enumerate((g - 1, g + 1)):
                        sl = s_c3(sig)
                        for kx in range(3):
                            mv = c3[:, sl:sl + 4, kx:kx + 256]
                            mv = mv.rearrange("p (s t) c -> p t s c", t=2)
                            nc.tensor.matmul(p, wdr[f"w4{kx}{'ab'[i]}"], mv,
                                             start=(i == 0 and kx == 0),
                                             stop=False,
                                             perf_mode=DR, skip_group_check=True)
                    for rho in range(2):
                        nc.tensor.matmul(
                            p[rho * D:(rho + 1) * D, :], ii_t,
                            oy[:, oslot(g + rho):oslot(g + rho) + 4:2, :],
                            start=False, stop=(rho == 1), skip_group_check=True)
                    # one 128-partition copy (cost scales with free size only),
                    # then one DMA whose 4D dram AP undoes the parity interleave
                    so = outs.tile([2 * D, 512], F32, tag="so")
                    n = so.free_size()
                    ca, cd = n * 0.833 + 143.0, n * 1.042 + 125.0
                    if acc[0] + ca <= acc[1] + cd:
                        acc[0] += ca
                        nc.scalar.activation(out=so, in_=p, func=AF.Copy,
                                             bias=0.0, scale=1.0)
                    else:
                        acc[1] += cd
                        nc.vector.tensor_copy(out=so, in_=p)
                    for rho in range(2):
                        nc.sync.dma_start(
                            out=out_d[:, g + rho:g + rho + 3:2, :],
                            in_=so[rho * D:(rho + 1) * D, :])
    nc.compile()
    return nc


_NC_CACHE = {}


def _get_ncs():
    if "a" not in _NC_CACHE:
        _NC_CACHE["a"] = _build_nc_a()
        _NC_CACHE["b"] = _build_nc_b()
    return _NC_CACHE["a"], _NC_CACHE["b"]


def _host_fold(G, Sx, Sy, Wq, bq, Wk, bk, Vw, vb):
    """Raw Gram [128,128] + channel sums -> (Wav [64,64], bav [64]) in f64."""
    G = G.astype(np.float64)
    Gxx, Gxy, Gyy = G[:D, :D], G[:D, D:], G[D:, D:]
    n = float(HW)
    QK = (Wq @ Gxy @ Wk.T + np.outer(Wq @ Sx, bk)
          + np.outer(bq, Wk @ Sy) + n * np.outer(bq, bk))
    qq = np.einsum('ij,jk,ik->i', Wq, Gxx, Wq) + 2 * bq * (Wq @ Sx) + n * bq * bq
    kk = np.einsum('ij,jk,ik->i', Wk, Gyy, Wk) + 2 * bk * (Wk @ Sy) + n * bk * bk
    St = QK / np.maximum(np.sqrt(qq), 1e-12)[:, None] \
            / np.maximum(np.sqrt(kk), 1e-12)[None, :]
    A = np.zeros((D, D))
    for h in range(4):
        blk = St[16 * h:16 * h + 16, 16 * h:16 * h + 16]
        e = np.exp(blk - blk.max(axis=1, keepdims=True))
        A[16 * h:16 * h + 16, 16 * h:16 * h + 16] = e / e.sum(axis=1, keepdims=True)
    return A @ Vw, A @ vb


def _prep_dr_weights(w):
    """w [64o, 64i, 3, 3] f32 -> dict kx -> (Wa, Wb) [64, 2, 128] e4m3."""
    out = {}
    for kx in range(3):
        Wa = np.zeros((D, 2, 2 * D), np.float32)
        Wb = np.zeros((D, 2, 2 * D), np.float32)
        wt = w[:, :, :, kx]  # [o, i, ky]
        Wa[:, 0, 0:D] = wt[:, :, 0].T
        Wa[:, 1, 0:D] = wt[:, :, 1].T
        Wa[:, 1, D:2 * D] = wt[:, :, 0].T
        Wb[:, 0, 0:D] = wt[:, :, 2].T
        Wb[:, 0, D:2 * D] = wt[:, :, 1].T
        Wb[:, 1, D:2 * D] = wt[:, :, 2].T
        out[kx] = (Wa.astype(E4), Wb.astype(E4))
    return out


def kernel(x, y, qw, qb, kw, kb, vw, vb,
           r1w1, r1b1, r1w2, r1b2, r2w1, r2b1, r2w2, r2b2, **_):
    x = np.asarray(x, np.float32)
    y = np.asarray(y, np.float32)
    qw, qb, kw, kb = (np.asarray(a, np.float32) for a in (qw, qb, kw, kb))
    vw, vb = np.asarray(vw, np.float32), np.asarray(vb, np.float32)
    r1w1, r1b1, r1w2, r1b2 = (np.asarray(a, np.float32) for a in (r1w1, r1b1, r1w2, r1b2))
    r2w1, r2b1, r2w2, r2b2 = (np.asarray(a, np.float32) for a in (r2w1, r2b1, r2w2, r2b2))
    nca, ncb = _get_ncs()

    # ---- pass A: pixel-major Gram
    in_maps_a = []
    xs_l, ys_l = [], []
    for c in range(NCORES):
        xc = x[c].reshape(D, HW)
        yc = y[c].reshape(D, HW)
        xs_l.append(xc.sum(axis=1, dtype=np.float64))
        ys_l.append(yc.sum(axis=1, dtype=np.float64))
        Z = np.empty((HW, 2 * D), np.float32)
        Z[:, :D] = xc.T
        Z[:, D:] = yc.T
        za = Z.reshape(32, 16, 128, 128).transpose(0, 2, 1, 3) \
              .reshape(32, 128, 2048).astype(BF)
        in_maps_a.append({"za": np.ascontiguousarray(za)})
    res_a = run_bass_kernel_spmd(nca, in_maps_a, core_ids=list(range(NCORES)))

    # ---- host fold + pass-B constants
    Wq, Wk, Vw = qw[:, :, 0, 0].astype(np.float64), kw[:, :, 0, 0].astype(np.float64), \
        vw[:, :, 0, 0].astype(np.float64)
    bq64, bk64, vb64 = qb.astype(np.float64), kb.astype(np.float64), vb.astype(np.float64)
    wdr_np = {}
    for ci, w in ((1, r1w1), (2, r1w2), (3, r2w1), (4, r2w2)):
        d = _prep_dr_weights(w)
        for kx in range(3):
            wdr_np[f"w{ci}{kx}a"] = d[kx][0]
            wdr_np[f"w{ci}{kx}b"] = d[kx][1]
    ii = np.concatenate([np.eye(D, dtype=np.float32)] * 2, axis=0).astype(BF)

    in_maps_b = []
    for c in range(NCORES):
        Wav, bav = _host_fold(res_a.results[c]["gout"], xs_l[c], ys_l[c],
                              Wq, bq64, Wk, bk64, Vw, vb64)
        m = {"yb": np.ascontiguousarray(y[c].reshape(D, H, W_IMG).astype(BF)),
             "wavt": np.ascontiguousarray(Wav.T.astype(np.float32).astype(BF)),
             "ii": ii,
             "bt0": bav.astype(np.float32).reshape(D, 1),
             "bc1": r1b1.reshape(D, 1),
             "bo3b": (bav + r1b2 + r2b2).astype(np.float32).reshape(D, 1),
             "bn4": (-r2b2).astype(np.float32).reshape(D, 1),
             "bc3": r2b1.reshape(D, 1)}
        m.update(wdr_np)
        in_maps_b.append({k: np.ascontiguousarray(v) for k, v in m.items()})
    res_b = run_bass_kernel_spmd(ncb, in_maps_b, core_ids=list(range(NCORES)))

    return np.stack([res_b.results[c]["out"].reshape(D, H, W_IMG)
                     for c in range(NCORES)]).astype(np.float32)


if __name__ == "__main__":
    rng = np.random.default_rng(0)
    ins = {
        "x": rng.standard_normal((8, D, H, W_IMG)).astype(np.float32),
        "y": rng.standard_normal((8, D, H, W_IMG)).astype(np.float32),
        "qw": (rng.standard_normal((D, D, 1, 1)) / 8).astype(np.float32),
        "qb": (rng.standard_normal(D) / 8).astype(np.float32),
        "kw": (rng.standard_normal((D, D, 1, 1)) / 8).astype(np.float32),
        "kb": (rng.standard_normal(D) / 8).astype(np.float32),
        "vw": (rng.standard_normal((D, D, 1, 1)) / 8).astype(np.float32),
        "vb": (rng.standard_normal(D) / 8).astype(np.float32),
    }
    for i in (1, 2):
        for j in (1, 2):
            ins[f"r{i}w{j}"] = (rng.standard_normal((D, D, 3, 3)) / 24).astype(np.float32)
            ins[f"r{i}b{j}"] = (rng.standard_normal(D) / 24).astype(np.float32)
    o = kernel(**ins)
    print("kernel ran, out shape", o.shape, "std", o.std())


# revision 19
# speedup vs baseline: 2.4925x; 1.0000x over previous
"""Trainium2 Bass kernel for channel-attention + 2 residual conv blocks.

Data-parallel over batch (8 cores, 1 batch element each). Two SPMD launches:
  A) raw channel Gram G = [x;y]^T-pixel-contraction ([128,128]) via 512
     accumulating matmuls on pixel-major data (host pre-transposes); no
     per-tile PSUM->SBUF copies at all.
  B) fused attention-apply + 4 3x3 convs. Convs run in fp8-e4m3 DoubleRow
     matmuls (2 k-tiles replace the row-shifted duplicate half entirely),
     packed 4 output rows per matmul group (M=128=2rows x 64ch,
     N=512=2 slot-pairs x 256 cols). Residual paths stay exact via bf16
     identity/Wav injection matmuls into PSUM; out rows DMA straight from
     PSUM. Host does only the O(64^2) softmax/fold algebra between launches.
"""
import sys, os
for p in ('/opt/trn_rl_repo', os.path.expanduser('~/.axon_site/_ro/trn_rl_repo')):
    if os.path.isdir(p) and p not in sys.path:
        sys.path.insert(0, p)

import numpy as np
import ml_dtypes
import concourse.bass as bass
import concourse.bacc as bacc
import concourse.tile as tile
from concourse import mybir
from concourse.bass_utils import run_bass_kernel_spmd

dt = mybir.dt
F32, BF16, FP8 = dt.float32, dt.bfloat16, dt.float8e4
BF = ml_dtypes.bfloat16
E4 = ml_dtypes.float8_e4m3
AF = mybir.ActivationFunctionType
OP = mybir.AluOpType
DR = mybir.MatmulPerfMode.DoubleRow

D = 64
HW = 65536
H = W_IMG = 256
NCORES = 8
R = 32  # output rows per block in pass B


def _build_nc_a():
    nc = bacc.Bacc("TRN2", target_bir_lowering=False, debug=False)
    za = nc.dram_tensor("za", [32, 128, 2048], BF16, kind="ExternalInput").ap()
    gout = nc.dram_tensor("gout", [128, 128], F32, kind="ExternalOutput").ap()
    with tile.TileContext(nc) as tc:
        with tc.tile_pool(name="io", bufs=3) as io, \
             tc.tile_pool(name="work", bufs=1) as work, \
             tc.tile_pool(name="gps", bufs=1, space="PSUM") as gps:
            gp = gps.tile([128, 128], F32)
            for t in range(32):
                zt = io.tile([128, 2048], BF16, tag="zt")
                nc.sync.dma_start(out=zt, in_=za[t])
                for j in range(16):
                    s = zt[:, j * 128:(j + 1) * 128]
                    nc.tensor.matmul(gp, s, s,
                                     start=(t == 0 and j == 0),
                                     stop=(t == 31 and j == 15),
                                     skip_group_check=True)
            gs = work.tile([128, 128], F32)
            nc.vector.tensor_copy(out=gs, in_=gp)
            nc.sync.dma_start(out=gout, in_=gs)
    nc.compile()
    return nc


def _emit_groups(lo, hi):
    """4-row groups (+2-row remainder; odd counts overlap by one row)."""
    out, g, n = [], lo, hi - lo
    while n >= 4:
        out.append((g, 4)); g += 4; n -= 4
    if n == 3:
        out.append((hi - 4, 4))
    elif n == 2:
        out.append((g, 2))
    elif n == 1:
        out.append((hi - 2, 2))
    return out


def _build_nc_b():
    nc = bacc.Bacc("TRN2", target_bir_lowering=False, debug=False)
    yb = nc.dram_tensor("yb", [D, H, W_IMG], BF16, kind="ExternalInput").ap()
    wavt = nc.dram_tensor("wavt", [D, D], BF16, kind="ExternalInput").ap()
    ii_d = nc.dram_tensor("ii", [2 * D, D], BF16, kind="ExternalInput").ap()
    # fp8 DoubleRow weights: per conv, per kx, (a|b) variant [64, 2, 128]
    wdr_d = {}
    for c in range(1, 5):
        for kx in range(3):
            for v in 'ab':
                nm = f"w{c}{kx}{v}"
                wdr_d[nm] = nc.dram_tensor(nm, [D, 2, 2 * D], FP8,
                                           kind="ExternalInput").ap()
    bias_d = {nm: nc.dram_tensor(nm, [D, 1], F32, kind="ExternalInput").ap()
              for nm in ('bt0', 'bc1', 'bo3b', 'bn4', 'bc3')}
    out_d = nc.dram_tensor("out", [D, H, W_IMG], F32, kind="ExternalOutput").ap()

    with tile.TileContext(nc) as tc:
        with tc.tile_pool(name="consts", bufs=1) as consts, \
             tc.tile_pool(name="stg", bufs=1) as stg, \
             tc.tile_pool(name="stgr", bufs=2) as stgr, \
             tc.tile_pool(name="oyp", bufs=2) as oyp, \
             tc.tile_pool(name="outs", bufs=4) as outs, \
             tc.tile_pool(name="ps", bufs=6, space="PSUM") as ps, \
             tc.tile_pool(name="ps2", bufs=2, space="PSUM") as ps2:
            wavt_t = consts.tile([D, D], BF16)
            ii_t = consts.tile([2 * D, D], BF16)
            nc.sync.dma_start(out=wavt_t, in_=wavt)
            nc.sync.dma_start(out=ii_t, in_=ii_d)
            wdr = {}
            for nm, d in wdr_d.items():
                t = consts.tile([D, 2, 2 * D], FP8, tag=nm)
                nc.sync.dma_start(out=t, in_=d)
                wdr[nm] = t
            bias = {}
            for nm, d in bias_d.items():
                t = consts.tile([D, 1], F32, tag=nm)
                nc.sync.dma_start(out=t, in_=d)
                bias[nm] = t

            # t0 persists across blocks (full image); col 0/257 stay zero
            t0 = stg.tile([D, 258, 258], FP8)   # slot = row + 1
            nc.vector.memset(t0[:, :, 0:1], 0.0)
            nc.vector.memset(t0[:, :, 257:258], 0.0)
            nc.vector.memset(t0[:, 0:1, :], 0.0)     # virtual row -1
            nc.vector.memset(t0[:, 257:258, :], 0.0)  # virtual row 256

            # stage-write engine balance: psum -> stage with bias (+relu).
            # GPSIMD cannot read PSUM, so these go to ACT/DVE, weighted by
            # modeled per-op cost (ACT 0.833ns/el +143, DVE 1.042ns/el +125).
            acc = [0.0, 0.0]

            def wr(out_ap, in_ap, b, relu):
                n = out_ap.free_size()
                ca, cd = n * 0.833 + 143.0, n * 1.042 + 125.0
                if acc[0] + ca <= acc[1] + cd:
                    acc[0] += ca
                    nc.scalar.activation(out=out_ap, in_=in_ap,
                                         func=(AF.Relu if relu else AF.Identity),
                                         bias=b, scale=1.0)
                elif relu:
                    acc[1] += cd
                    nc.vector.tensor_scalar(out=out_ap, in0=in_ap, scalar1=b,
                                            scalar2=0.0, op0=OP.add, op1=OP.max)
                else:
                    acc[1] += cd
                    nc.vector.tensor_scalar_add(out=out_ap, in0=in_ap, scalar1=b)

            # --- block loop -------------------------------------------------
            t0_done = 0  # t0 rows produced so far
            for blk in range(8):
                r0, r1 = blk * R, (blk + 1) * R
                oy = oyp.tile([2 * D, 44, W_IMG], BF16, tag="oy")
                # per-block stages, double-buffered for cross-block overlap
                c1 = stgr.tile([D, 40, 258], FP8, tag="c1")
                o3f = stgr.tile([D, 38, 258], FP8, tag="o3f")
                c3 = stgr.tile([D, 36, 258], FP8, tag="c3")
                if blk < 2:  # each rotating buffer's gap cols, zeroed once
                    for t in (c1, o3f, c3):
                        nc.vector.memset(t[:, :, 0:1], 0.0)
                        nc.vector.memset(t[:, :, 257:258], 0.0)

                def oslot(row):
                    return row - (r0 - 4)

                ylo, yhi = max(r0 - 4, 0), min(r1 + 4, 256)
                nc.sync.dma_start(out=oy[0:D, oslot(ylo):oslot(yhi), :],
                                  in_=yb[:, ylo:yhi, :])

                # stage ranges (produced rows)
                c1lo, c1hi = max(r0 - 3, 0), min(r1 + 3, 256)
                o3lo, o3hi = max(r0 - 2, 0), min(r1 + 2, 256)
                c3lo, c3hi = max(r0 - 1, 0), min(r1 + 1, 256)

                def s_c1(row): return row - c1lo + 1
                def s_o3(row): return row - o3lo + 1
                def s_c3(row): return row - c3lo + 1
                def s_t0(row): return row + 1

                # virtual zero rows at image edges (persistent tiles: emit
                # only when the slot is actually consumed as a virtual row)
                if blk == 0:
                    for t in (c1, o3f, c3):
                        nc.vector.memset(t[:, 0:1, :], 0.0)
                if blk == 7:
                    nc.vector.memset(c1[:, s_c1(256):s_c1(256) + 1, :], 0.0)
                    nc.vector.memset(o3f[:, s_o3(256):s_o3(256) + 1, :], 0.0)
                    nc.vector.memset(c3[:, s_c3(256):s_c3(256) + 1, :], 0.0)

                # ---- t0 (= Wav y + bav), full-image persistent, 4-row steps
                t0_hi = min(r1 + 4, 256)
                for g in range(t0_done, t0_hi, 2):
                    p2 = ps2.tile([D, 512], F32, tag="p2")
                    nc.tensor.matmul(p2, wavt_t,
                                     oy[0:D, oslot(g):oslot(g) + 2, :],
                                     start=True, stop=True, skip_group_check=True)
                    wr(t0[:, s_t0(g):s_t0(g) + 2, 1:257], p2, bias['bt0'], False)
                t0_done = t0_hi

                # ---- conv1: t0 -> c1 (relu, b1)
                for g, sz in _emit_groups(c1lo, c1hi):
                    ssz = sz // 2
                    p = ps.tile([2 * D, 128 * sz], F32, tag="cv")
                    last = None
                    for i, sig in enumerate((g - 1, g + 1)):
                        sl = s_t0(sig)
                        for kx in range(3):
                            mv = t0[:, sl:sl + 2 * ssz, kx:kx + 256]
                            mv = mv.rearrange("p (s t) c -> p t s c", t=2)
                            nc.tensor.matmul(p, wdr[f"w1{kx}{'ab'[i]}"], mv,
                                             start=(i == 0 and kx == 0),
                                             stop=(i == 1 and kx == 2),
                                             perf_mode=DR, skip_group_check=True)
                    for rho in range(2):
                        dst = c1[:, s_c1(g + rho):s_c1(g + rho) + 2 * ssz:2, 1:257]
                        wr(dst, p[rho * D:(rho + 1) * D, :], bias['bc1'], True)

                # ---- conv2 + Wav-inject: c1 -> o3 (fp8 + bf16-in-oy)
                for g, sz in _emit_groups(o3lo, o3hi):
                    ssz = sz // 2
                    p = ps.tile([2 * D, 128 * sz], F32, tag="cv")
                    for i, sig in enumerate((g - 1, g + 1)):
                        sl = s_c1(sig)
                        for kx in range(3):
                            mv = c1[:, sl:sl + 2 * ssz, kx:kx + 256]
                            mv = mv.rearrange("p (s t) c -> p t s c", t=2)
                            nc.tensor.matmul(p, wdr[f"w2{kx}{'ab'[i]}"], mv,
                                             start=(i == 0 and kx == 0),
                                             stop=False,
                                             perf_mode=DR, skip_group_check=True)
                    for rho in range(2):
                        nc.tensor.matmul(
                            p[rho * D:(rho + 1) * D, :], wavt_t,
                            oy[0:D, oslot(g + rho):oslot(g + rho) + 2 * ssz:2, :],
                            start=False, stop=(rho == 1), skip_group_check=True)
                    for rho in range(2):
                        psl = p[rho * D:(rho + 1) * D, :]
                        dstb = oy[D:2 * D, oslot(g + rho):oslot(g + rho) + 2 * ssz:2, :]
                        wr(dstb, psl, bias['bo3b'], False)
                        # fp8 copy for conv3 input: GPSIMD from the bf16 o3
                        # (o3_bf16 carries +b4; subtract it again here)
                        dst = o3f[:, s_o3(g + rho):s_o3(g + rho) + 2 * ssz:2, 1:257]
                        nc.gpsimd.tensor_scalar_add(out=dst, in0=dstb,
                                                    scalar1=bias['bn4'])

                # ---- conv3: o3f -> c3 (relu, b3)
                for g, sz in _emit_groups(c3lo, c3hi):
                    ssz = sz // 2
                    p = ps.tile([2 * D, 128 * sz], F32, tag="cv")
                    for i, sig in enumerate((g - 1, g + 1)):
                        sl = s_o3(sig)
                        for kx in range(3):
                            mv = o3f[:, sl:sl + 2 * ssz, kx:kx + 256]
                            mv = mv.rearrange("p (s t) c -> p t s c", t=2)
                            nc.tensor.matmul(p, wdr[f"w3{kx}{'ab'[i]}"], mv,
                                             start=(i == 0 and kx == 0),
                                             stop=(i == 1 and kx == 2),
                                             perf_mode=DR, skip_group_check=True)
                    for rho in range(2):
                        dst = c3[:, s_c3(g + rho):s_c3(g + rho) + 2 * ssz:2, 1:257]
                        wr(dst, p[rho * D:(rho + 1) * D, :], bias['bc3'], True)

                # ---- conv4 + (o3+b4+y)-inject: c3 -> out (DMA from PSUM)
                for g in range(r0, r1, 4):
                    p = ps.tile([2 * D, 512], F32, tag="cv")
                    for i, sig in enumerate((g - 1, g + 1)):
                        sl = s_c3(sig)
                        for kx in range(3):
                            mv = c3[:, sl:sl + 4, kx:kx + 256]
                            mv = mv.rearrange("p (s t) c -> p t s c", t=2)
                            nc.tensor.matmul(p, wdr[f"w4{kx}{'ab'[i]}"], mv,
                                             start=(i == 0 and kx == 0),
                                             stop=False,
                                             perf_mode=DR, skip_group_check=True)
                    for rho in range(2):
                        nc.tensor.matmul(
                            p[rho * D:(rho + 1) * D, :], ii_t,
                            oy[:, oslot(g + rho):oslot(g + rho) + 4:2, :],
                            start=False, stop=(rho == 1), skip_group_check=True)
                    # one 128-partition copy (cost scales with free size only),
                    # then one DMA whose 4D dram AP undoes the parity interleave
                    so = outs.tile([2 * D, 512], F32, tag="so")
                    n = so.free_size()
                    ca, cd = n * 0.833 + 143.0, n * 1.042 + 125.0
                    if acc[0] + ca <= acc[1] + cd:
                        acc[0] += ca
                        nc.scalar.activation(out=so, in_=p, func=AF.Copy,
                                             bias=0.0, scale=1.0)
                    else:
                        acc[1] += cd
                        nc.vector.tensor_copy(out=so, in_=p)
                    for rho in range(2):
                        nc.sync.dma_start(
                            out=out_d[:, g + rho:g + rho + 3:2, :],
                            in_=so[rho * D:(rho + 1) * D, :])
    nc.compile()
    return nc


_NC_CACHE = {}


def _get_ncs():
    if "a" not in _NC_CACHE:
        _NC_CACHE["a"] = _build_nc_a()
        _NC_CACHE["b"] = _build_nc_b()
    return _NC_CACHE["a"], _NC_CACHE["b"]


def _host_fold(G, Sx, Sy, Wq, bq, Wk, bk, Vw, vb):
    """Raw Gram [128,128] + channel sums -> (Wav [64,64], bav [64]) in f64."""
    G = G.astype(np.float64)
    Gxx, Gxy, Gyy = G[:D, :D], G[:D, D:], G[D:, D:]
    n = float(HW)
    QK = (Wq @ Gxy @ Wk.T + np.outer(Wq @ Sx, bk)
          + np.outer(bq, Wk @ Sy) + n * np.outer(bq, bk))
    qq = np.einsum('ij,jk,ik->i', Wq, Gxx, Wq) + 2 * bq * (Wq @ Sx) + n * bq * bq
    kk = np.einsum('ij,jk,ik->i', Wk, Gyy, Wk) + 2 * bk * (Wk @ Sy) + n * bk * bk
    St = QK / np.maximum(np.sqrt(qq), 1e-12)[:, None] \
            / np.maximum(np.sqrt(kk), 1e-12)[None, :]
    A = np.zeros((D, D))
    for h in range(4):
        blk = St[16 * h:16 * h + 16, 16 * h:16 * h + 16]
        e = np.exp(blk - blk.max(axis=1, keepdims=True))
        A[16 * h:16 * h + 16, 16 * h:16 * h + 16] = e / e.sum(axis=1, keepdims=True)
    return A @ Vw, A @ vb


def _prep_dr_weights(w):
    """w [64o, 64i, 3, 3] f32 -> dict kx -> (Wa, Wb) [64, 2, 128] e4m3."""
    out = {}
    for kx in range(3):
        Wa = np.zeros((D, 2, 2 * D), np.float32)
        Wb = np.zeros((D, 2, 2 * D), np.float32)
        wt = w[:, :, :, kx]  # [o, i, ky]
        Wa[:, 0, 0:D] = wt[:, :, 0].T
        Wa[:, 1, 0:D] = wt[:, :, 1].T
        Wa[:, 1, D:2 * D] = wt[:, :, 0].T
        Wb[:, 0, 0:D] = wt[:, :, 2].T
        Wb[:, 0, D:2 * D] = wt[:, :, 1].T
        Wb[:, 1, D:2 * D] = wt[:, :, 2].T
        out[kx] = (Wa.astype(E4), Wb.astype(E4))
    return out


def kernel(x, y, qw, qb, kw, kb, vw, vb,
           r1w1, r1b1, r1w2, r1b2, r2w1, r2b1, r2w2, r2b2, **_):
    x = np.asarray(x, np.float32)
    y = np.asarray(y, np.float32)
    qw, qb, kw, kb = (np.asarray(a, np.float32) for a in (qw, qb, kw, kb))
    vw, vb = np.asarray(vw, np.float32), np.asarray(vb, np.float32)
    r1w1, r1b1, r1w2, r1b2 = (np.asarray(a, np.float32) for a in (r1w1, r1b1, r1w2, r1b2))
    r2w1, r2b1, r2w2, r2b2 = (np.asarray(a, np.float32) for a in (r2w1, r2b1, r2w2, r2b2))
    nca, ncb = _get_ncs()

    # ---- pass A: pixel-major Gram
    in_maps_a = []
    xs_l, ys_l = [], []
    for c in range(NCORES):
        xc = x[c].reshape(D, HW)
        yc = y[c].reshape(D, HW)
        xs_l.append(xc.sum(axis=1, dtype=np.float64))
        ys_l.append(yc.sum(axis=1, dtype=np.float64))
        Z = np.empty((HW, 2 * D), np.float32)
        Z[:, :D] = xc.T
        Z[:, D:] = yc.T
        za = Z.reshape(32, 16, 128, 128).transpose(0, 2, 1, 3) \
              .reshape(32, 128, 2048).astype(BF)
        in_maps_a.append({"za": np.ascontiguousarray(za)})
    res_a = run_bass_kernel_spmd(nca, in_maps_a, core_ids=list(range(NCORES)))

    # ---- host fold + pass-B constants
    Wq, Wk, Vw = qw[:, :, 0, 0].astype(np.float64), kw[:, :, 0, 0].astype(np.float64), \
        vw[:, :, 0, 0].astype(np.float64)
    bq64, bk64, vb64 = qb.astype(np.float64), kb.astype(np.float64), vb.astype(np.float64)
    wdr_np = {}
    for ci, w in ((1, r1w1), (2, r1w2), (3, r2w1), (4, r2w2)):
        d = _prep_dr_weights(w)
        for kx in range(3):
            wdr_np[f"w{ci}{kx}a"] = d[kx][0]
            wdr_np[f"w{ci}{kx}b"] = d[kx][1]
    ii = np.concatenate([np.eye(D, dtype=np.float32)] * 2, axis=0).astype(BF)

    in_maps_b = []
    for c in range(NCORES):
        Wav, bav = _host_fold(res_a.results[c]["gout"], xs_l[c], ys_l[c],
                              Wq, bq64, Wk, bk64, Vw, vb64)
        m = {"yb": np.ascontiguousarray(y[c].reshape(D, H, W_IMG).astype(BF)),
             "wavt": np.ascontiguousarray(Wav.T.astype(np.float32).astype(BF)),
             "ii": ii,
             "bt0": bav.astype(np.float32).reshape(D, 1),
             "bc1": r1b1.reshape(D, 1),
             "bo3b": (bav + r1b2 + r2b2).astype(np.float32).reshape(D, 1),
             "bn4": (-r2b2).astype(np.float32).reshape(D, 1),
             "bc3": r2b1.reshape(D, 1)}
        m.update(wdr_np)
        in_maps_b.append({k: np.ascontiguousarray(v) for k, v in m.items()})
    res_b = run_bass_kernel_spmd(ncb, in_maps_b, core_ids=list(range(NCORES)))

    return np.stack([res_b.results[c]["out"].reshape(D, H, W_IMG)
                     for c in range(NCORES)]).astype(np.float32)


if __name__ == "__main__":
    rng = np.random.default_rng(0)
    ins = {
        "x": rng.standard_normal((8, D, H, W_IMG)).astype(np.float32),
        "y": rng.standard_normal((8, D, H, W_IMG)).astype(np.float32),
        "qw": (rng.standard_normal((D, D, 1, 1)) / 8).astype(np.float32),
        "qb": (rng.standard_normal(D) / 8).astype(np.float32),
        "kw": (rng.standard_normal((D, D, 1, 1)) / 8).astype(np.float32),
        "kb": (rng.standard_normal(D) / 8).astype(np.float32),
        "vw": (rng.standard_normal((D, D, 1, 1)) / 8).astype(np.float32),
        "vb": (rng.standard_normal(D) / 8).astype(np.float32),
    }
    for i in (1, 2):
        for j in (1, 2):
            ins[f"r{i}w{j}"] = (rng.standard_normal((D, D, 3, 3)) / 24).astype(np.float32)
            ins[f"r{i}b{j}"] = (rng.standard_normal(D) / 24).astype(np.float32)
    o = kernel(**ins)
    print("kernel ran, out shape", o.shape, "std", o.std())
